# revision 16
# baseline (speedup 1.0000x reference)
"""MultiHeadAttention TRN2 Bass kernel.

Problem: B=4, S=2048, D=768, H=12 heads (DK=64).
Sharding: 8 cores = (batch b in 0..3) x (head-half in 0..1); each core
computes 6 heads of one batch element end-to-end (tensor-parallel over
heads within a batch). Host pre-transposes activations to [D, S] (and
casts to bf16 in the default fast path), slices projection weights per
head-half, and sums the two partial outputs per batch (+ bv@Wo + bo
correction, exact because softmax rows sum to 1).

On-core math:
  qh^T[384, S]: lhsT=Wq tile [Din,dout], rhs=q^T tile [Din,s] (+bq in drain)
  kh^T likewise; vh natural [S, 390] via lhsT=v^T tile, rhs=Wv:
    vh_aug[s, 65j..65j+64] = [m(s)*vh_head_j(s, :), m(s)]  (mask fold)
  S^T[k, q] = kh_head^T.T @ qh_head^T  (contraction d=64)
  P^T = exp(S^T * 0.125)               (ACT, fused scale, no max-sub)
  ctx_aug^T[0:65, q] += vh_aug_j[kc].T @ P^T[kc]  over k-chunks
    rows 0..63 = unnormalized ctx^T, row 64 = softmax denominator
  rs = approx-recip(denom); bcast via ones[1,64] K=1 f32r matmul;
  cn = ctx^T * rs   (drains deferred one (h,qb) iteration so the PE
                     pipeline never waits on the reciprocal chain)
  out[q, 768] = sum_dt cn[dt].T @ Wo tiles  (per 128-q chunk)
"""

import os
import sys
import types
from contextlib import ExitStack

import ml_dtypes
import numpy as np

import concourse.bacc as bacc
import concourse.bass as bass
import concourse.mybir as mybir
import concourse.tile as tile
from concourse import bass_utils
from concourse.bass import ts, ds

F32 = mybir.dt.float32
F32R = mybir.dt.float32r
BF16 = mybir.dt.bfloat16

D = 768        # model dim
DH = 384       # per-core head dim (6 heads x 64)
HPC = 6        # heads per core
VW = HPC * 65  # vh_aug free width (390)
QB = 1024      # q-block width in attention phase


def build_nc(S=2048, bf16=True):
    nc = bacc.Bacc("TRN2", target_bir_lowering=False, debug=False)

    MMD = BF16 if bf16 else F32R    # matmul operand dtype
    QBW = min(QB, S)                # attention q-block width
    NKT = S // 128                  # 128-wide s/k tiles
    NQB = S // QBW                  # q blocks
    NSC = S // 512                  # 512-wide s chunks

    qT = nc.dram_tensor("qT", [D, S], MMD, kind="ExternalInput").ap()
    kT = nc.dram_tensor("kT", [D, S], MMD, kind="ExternalInput").ap()
    vT = nc.dram_tensor("vT", [D, S], MMD, kind="ExternalInput").ap()
    wq = nc.dram_tensor("wq", [D, DH], MMD, kind="ExternalInput").ap()
    wk = nc.dram_tensor("wk", [D, DH], MMD, kind="ExternalInput").ap()
    wv = nc.dram_tensor("wv", [D, DH], MMD, kind="ExternalInput").ap()
    wo = nc.dram_tensor("wo", [DH, D], MMD, kind="ExternalInput").ap()
    bq = nc.dram_tensor("bq", [DH, 1], F32, kind="ExternalInput").ap()
    bk = nc.dram_tensor("bk", [DH, 1], F32, kind="ExternalInput").ap()
    mv = nc.dram_tensor("mv", [S, 1], F32, kind="ExternalInput").ap()
    out = nc.dram_tensor("out", [S, D], F32, kind="ExternalOutput").ap()

    with tile.TileContext(nc) as tc, ExitStack() as ctx:
        P = 128
        wpool = ctx.enter_context(tc.tile_pool(name="w", bufs=1))
        xin = ctx.enter_context(tc.tile_pool(name="xin", bufs=8))
        persist = ctx.enter_context(tc.tile_pool(name="persist", bufs=1))
        ppool = ctx.enter_context(tc.tile_pool(name="p", bufs=3))
        small = ctx.enter_context(tc.tile_pool(name="small", bufs=2))
        outp = ctx.enter_context(tc.tile_pool(name="outp", bufs=2))
        psA = ctx.enter_context(tc.tile_pool(name="psA", bufs=2, space="PSUM"))
        psB = ctx.enter_context(tc.tile_pool(name="psB", bufs=2, space="PSUM"))

        # ---- constants / small tensors ----
        wq_sb = [wpool.tile([P, DH], MMD, name=f"wq{c}", tag=f"wq{c}") for c in range(6)]
        wk_sb = [wpool.tile([P, DH], MMD, name=f"wk{c}", tag=f"wk{c}") for c in range(6)]
        wv_sb = [wpool.tile([P, DH], MMD, name=f"wv{c}", tag=f"wv{c}") for c in range(6)]
        wo_sb = [wpool.tile([P, D], MMD, name=f"wo{c}", tag=f"wo{c}") for c in range(3)]
        for c in range(6):
            (nc.sync if c % 2 == 0 else nc.gpsimd).dma_start(
                wk_sb[c][:], wk[ts(c, P), :]
            )
        bq_sb = [wpool.tile([P, 1], F32, name=f"bq{t}", tag=f"bq{t}") for t in range(3)]
        bk_sb = [wpool.tile([P, 1], F32, name=f"bk{t}", tag=f"bk{t}") for t in range(3)]
        for t in range(3):
            nc.sync.dma_start(bq_sb[t][:], bq[ts(t, P), :])
            nc.sync.dma_start(bk_sb[t][:], bk[ts(t, P), :])
        mv_sb = [wpool.tile([P, 1], F32, name=f"mv{st}", tag=f"mv{st}") for st in range(NKT)]
        for st in range(NKT):
            nc.sync.dma_start(mv_sb[st][:], mv[ts(st, P), :])
        ones6 = wpool.tile([P, HPC], F32, tag="ones6")
        nc.vector.memset(ones6[:], 1.0)
        ones64f = wpool.tile([1, 64], F32, tag="ones64f")
        nc.vector.memset(ones64f[:], 1.0)
        ones64 = wpool.tile([1, 64], F32R, tag="ones64")
        nc.vector.tensor_copy(ones64[:], ones64f[:])

        # ---- persistent activations ----
        khT = [persist.tile([P, S], MMD, name=f"khT{t}", tag=f"khT{t}") for t in range(3)]
        qhT = [persist.tile([P, S], MMD, name=f"qhT{t}", tag=f"qhT{t}") for t in range(3)]
        vh = [persist.tile([P, VW], MMD, name=f"vh{st}", tag=f"vh{st}") for st in range(NKT)]
        cn = [persist.tile([P, S], MMD, name=f"cn{t}", tag=f"cn{t}") for t in range(3)]

        # ---- phase 1a/1b: projections (q-proj of the second q-block is
        # deferred into the attention stream as PE filler work) ----
        def proj_chunk(xdram, wsb, bsb, dst, sc, pools=("psA", "psA", "psA")):
            xt = [xin.tile([P, QBW], MMD, name="xin", tag="xin") for c in range(6)]
            for c in range(6):
                (nc.sync if c % 2 == 0 else nc.gpsimd).dma_start(
                    xt[c][:], xdram[ts(c, P), ts(sc, QBW)]
                )
            for dt in range(3):
                pool = psA if pools[dt] == "psA" else psB
                ps = pool.tile([P, QBW], F32, name="prps", tag=pools[dt])
                for u in range(QBW // 512):
                    for c in range(6):
                        nc.tensor.matmul(
                            ps[:, ts(u, 512)],
                            lhsT=wsb[c][:, ts(dt, P)],
                            rhs=xt[c][:, ts(u, 512)],
                            start=(c == 0),
                            stop=(c == 5),
                        )
                nc.vector.tensor_scalar_add(
                    out=dst[dt][:, ts(sc, QBW)], in0=ps[:],
                    scalar1=bsb[dt][:],
                )

        # k-projection (alternating psum pools for deeper rotation)
        for sc in range(S // QBW):
            proj_chunk(kT, wk_sb, bk_sb, khT, sc,
                       pools=("psA", "psB", "psA") if sc % 2 == 0 else ("psB", "psA", "psB"))

        # v-projection
        for c in range(6):
            nc.sync.dma_start(wv_sb[c][:], wv[ts(c, P), :])
        for stb in range(0, NKT, QBW // 128):
            vt = [xin.tile([P, QBW], MMD, name="xin", tag="xin") for c in range(6)]
            for c in range(6):
                nc.sync.dma_start(vt[c][:], vT[ts(c, P), ds(stb * 128, QBW)])
            for sj in range(QBW // 128):
                st = stb + sj
                pool, ptag = (psA, "psA") if sj % 2 == 0 else (psB, "psB")
                ps = pool.tile([P, QBW], F32, name="vps", tag=ptag)
                for c in range(6):
                    nc.tensor.matmul(
                        ps[:, :DH],
                        lhsT=vt[c][:, ts(sj, P)],
                        rhs=wv_sb[c][:],
                        start=(c == 0),
                        stop=(c == 5),
                    )
                vh3 = vh[st].rearrange("p (h c) -> p h c", c=65)
                nc.vector.tensor_scalar_mul(
                    out=vh3[:, :, 0:64],
                    in0=ps[:, :DH].rearrange("p (h c) -> p h c", c=64),
                    scalar1=mv_sb[st][:],
                )
                nc.vector.tensor_scalar_mul(
                    out=vh3[:, :, 64:65],
                    in0=ones6[:].rearrange("p (h c) -> p h c", c=1),
                    scalar1=mv_sb[st][:],
                )

        # q-projection: first q-block now, rest deferred into phase 2
        for c in range(6):
            nc.sync.dma_start(wq_sb[c][:], wq[ts(c, P), :])
        proj_chunk(qT, wq_sb, bq_sb, qhT, 0,
                   pools=("psA", "psB", "psA"))
        for c in range(3):
            nc.sync.dma_start(wo_sb[c][:], wo[ts(c, P), :])
        pend_qproj = [(sc, dt) for sc in range(1, S // QBW) for dt in range(3)]
        qproj_xt = {}

        # ---- phase 2: attention ----
        # Flat global pipeline over (h, qb, kc) steps: scores run 2 steps
        # ahead of attn@V; N=1024 single matmuls keep PE duty high so the
        # HAM clock stays unthrottled. Drains (recip + gpsimd broadcast)
        # are deferred ~5us so the PE never waits on them.
        hq = [(h, qb) for qb in range(NQB) for h in range(HPC)]
        steps = [(h, qb, kc) for (h, qb) in hq for kc in range(NKT)]

        ctx_ps = {}
        st_ps = {}

        def scores(h, qb, kc):
            dt, pb = h // 2, 64 * (h % 2)
            ps = psA.tile([P, QBW], F32, name="psA", tag="psA")
            for u in range(QBW // 512):
                nc.tensor.matmul(
                    ps[:, ts(u, 512)],
                    lhsT=khT[dt][pb : pb + 64, ts(kc, P)],
                    rhs=qhT[dt][pb : pb + 64, ds(qb * QBW + u * 512, 512)],
                    start=True,
                    stop=True,
                )
            st_ps[(h, qb, kc)] = ps

        def attnv(h, qb, kc, pt):
            for u in range(QBW // 512):
                nc.tensor.matmul(
                    ctx_ps[(h, qb)][0:65, ts(u, 512)],
                    lhsT=vh[kc][:, ds(65 * h, 65)],
                    rhs=pt[:, ts(u, 512)],
                    start=(kc == 0),
                    stop=(kc == NKT - 1),
                )

        def drain(h, qb):
            """Normalize + store ctx for a finished (h, qb)."""
            dt, pb = h // 2, 64 * (h % 2)
            cps = ctx_ps.pop((h, qb))
            rs = small.tile([1, QBW], F32, name="rs", tag="rs")
            with nc.allow_low_precision(reason="softmax denom recip"):
                nc.vector.reciprocal(rs[:], cps[64:65, :])
            bcs = small.tile([64, QBW], F32, name="bcs", tag="bcs")
            nc.gpsimd.partition_broadcast(bcs[:], rs[:])
            if pb == 0:
                nc.vector.tensor_tensor(
                    out=cn[dt][0:64, ts(qb, QBW)],
                    in0=cps[0:64, :],
                    in1=bcs[:],
                    op=mybir.AluOpType.mult,
                )
            else:
                tmp = small.tile([64, QBW], MMD, name="tmp", tag="tmp")
                nc.vector.tensor_tensor(
                    out=tmp[:], in0=cps[0:64, :], in1=bcs[:],
                    op=mybir.AluOpType.mult,
                )
                nc.sync.dma_start(cn[dt][64:128, ts(qb, QBW)], tmp[:])

        def oproj(qc):
            ups = psB.tile([P, QBW], F32, name="oups", tag="psB")
            ups2 = ups if QBW >= D else psA.tile([P, 512], F32, name="psA", tag="psA")
            for ps_, n0, nw in ((ups, 0, 512), (ups2, 512, 256)):
                off = n0 if ps_ is ups else 0
                for dt in range(3):
                    nc.tensor.matmul(
                        ps_[:, ds(off, nw)] if ps_ is not ups else ps_[:, ds(n0, nw)],
                        lhsT=cn[dt][:, ts(qc, P)],
                        rhs=wo_sb[dt][:, ds(n0, nw)],
                        start=(dt == 0),
                        stop=(dt == 2),
                    )
            ot = outp.tile([P, D], F32, name="ot", tag="ot")
            if QBW >= D:
                nc.vector.tensor_copy(ot[:], ups[:, 0:D])
            else:
                nc.vector.tensor_copy(ot[:, 0:512], ups[:, 0:512])
                nc.vector.tensor_copy(ot[:, 512:D], ups2[:, 0:256])
            nc.sync.dma_start(out[ts(qc, P), :], ot[:])

        DEPTH = 2
        pend_drain = []
        pend_oproj = []
        for n, (h, qb, kc) in enumerate(steps):
            if kc == 0:
                ctx_ps[(h, qb)] = psB.tile([P, QBW], F32, name="psB", tag="psB")[0:65, :]
            if n < DEPTH:
                scores(*steps[n])
            pt = ppool.tile([P, QBW], MMD, name="pt", tag="pt")
            nc.scalar.activation(
                pt[:], st_ps.pop((h, qb, kc))[:],
                mybir.ActivationFunctionType.Exp, scale=0.125,
            )
            if n + DEPTH < len(steps):
                scores(*steps[n + DEPTH])
            attnv(h, qb, kc, pt)
            if kc == 2 and pend_drain:
                hd, qd = pend_drain.pop(0)
                drain(hd, qd)
                if hd == HPC - 1:
                    pend_oproj.extend(range(qd * (QBW // P), (qd + 1) * (QBW // P)))
            elif pend_oproj and (kc == 10 or (kc == 13 and len(pend_oproj) > 3)):
                oproj(pend_oproj.pop(0))
            elif pend_qproj and kc == 12:
                sc, dt = pend_qproj.pop(0)
                if sc not in qproj_xt:
                    qproj_xt[sc] = [
                        xin.tile([P, QBW], MMD, name="xin", tag="xin")
                        for c in range(6)
                    ]
                    for c in range(6):
                        nc.sync.dma_start(
                            qproj_xt[sc][c][:], qT[ts(c, P), ts(sc, QBW)]
                        )
                ps = psB.tile([P, QBW], F32, name="qprps", tag="psB")
                for u in range(QBW // 512):
                    for c in range(6):
                        nc.tensor.matmul(
                            ps[:, ts(u, 512)],
                            lhsT=wq_sb[c][:, ts(dt, P)],
                            rhs=qproj_xt[sc][c][:, ts(u, 512)],
                            start=(c == 0),
                            stop=(c == 5),
                        )
                nc.vector.tensor_scalar_add(
                    out=qhT[dt][:, ts(sc, QBW)], in0=ps[:],
                    scalar1=bq_sb[dt][:],
                )
                if dt == 2:
                    qproj_xt.pop(sc)
            if kc == NKT - 1:
                pend_drain.append((h, qb))
        for hd, qd in pend_drain:
            drain(hd, qd)
            if hd == HPC - 1:
                pend_oproj.extend(range(qd * (QBW // P), (qd + 1) * (QBW // P)))
        for qc in pend_oproj:
            oproj(qc)

    nc.compile()
    return nc


_NC_CACHE = {}


def _get_nc(S, bf16=True):
    key = (S, bf16)
    if key not in _NC_CACHE:
        _NC_CACHE[key] = build_nc(S, bf16)
    return _NC_CACHE[key]


def _install_ntff_hook():
    try:
        mod = types.ModuleType("antenv.axon_hooks")
        state = {"hook": None}
        mod.set_axon_ntff_profile_hook = lambda h: state.__setitem__("hook", h)
        mod.get_axon_ntff_profile_hook = lambda: state["hook"]
        sys.modules["antenv.axon_hooks"] = mod
        from trn_agent_boot.trn_boot import _ntff_profile_via_ctypes

        mod.set_axon_ntff_profile_hook(
            _ntff_profile_via_ctypes("/opt/axon/libaxon_pjrt.so")
        )
        bass_utils.upload_artifacts = lambda tmpdir: "local://" + tmpdir
        return state["hook"] is not None
    except Exception:
        return False


def run_cores(in_maps, S=2048, bf16=True, profile=False):
    nc = _get_nc(S, bf16)
    trace = bool(profile) and _install_ntff_hook()
    res = bass_utils.run_bass_kernel_spmd(
        nc, in_maps, core_ids=list(range(len(in_maps))), trace=trace
    )
    return res


def make_in_maps(q, k, v, mask, Wq, bq, Wk, bk, Wv, Wo, bf16=True):
    B = q.shape[0]
    mmd = ml_dtypes.bfloat16 if bf16 else np.float32
    qT = np.ascontiguousarray(
        np.asarray(q, np.float32).transpose(0, 2, 1)).astype(mmd)
    kT = np.ascontiguousarray(
        np.asarray(k, np.float32).transpose(0, 2, 1)).astype(mmd)
    vT = np.ascontiguousarray(
        np.asarray(v, np.float32).transpose(0, 2, 1)).astype(mmd)
    mvec = (~np.asarray(mask).reshape(B, -1)).astype(np.float32)
    Wq, Wk, Wv, Wo = (np.asarray(a, np.float32) for a in (Wq, Wk, Wv, Wo))
    bq, bk = np.asarray(bq, np.float32), np.asarray(bk, np.float32)
    in_maps = []
    for b in range(B):
        for half in range(2):
            hs = slice(DH * half, DH * (half + 1))
            in_maps.append(
                {
                    "qT": qT[b],
                    "kT": kT[b],
                    "vT": vT[b],
                    "wq": np.ascontiguousarray(Wq[:, hs]).astype(mmd),
                    "wk": np.ascontiguousarray(Wk[:, hs]).astype(mmd),
                    "wv": np.ascontiguousarray(Wv[:, hs]).astype(mmd),
                    "wo": np.ascontiguousarray(Wo[hs, :]).astype(mmd),
                    "bq": np.ascontiguousarray(bq[hs]).reshape(DH, 1),
                    "bk": np.ascontiguousarray(bk[hs]).reshape(DH, 1),
                    "mv": np.ascontiguousarray(mvec[b]).reshape(-1, 1),
                }
            )
    return in_maps


def kernel(q, k, v, mask, Wq, bq, Wk, bk, Wv, bv, Wo, bo):
    q = np.asarray(q, np.float32)
    B, S, _ = q.shape
    bf16 = os.environ.get("BASS_PRECISE") != "1"
    in_maps = make_in_maps(q, k, v, mask, Wq, bq, Wk, bk, Wv, Wo, bf16=bf16)
    res = run_cores(
        in_maps, S=S, bf16=bf16, profile=os.environ.get("BASS_PROFILE") == "1"
    )
    if os.environ.get("BASS_PROFILE") == "1" and res.exec_time_ns is not None:
        print(f"HW exec time: {res.exec_time_ns} ns")
    cvec = (
        np.asarray(bv, np.float32) @ np.asarray(Wo, np.float32)
        + np.asarray(bo, np.float32)
    )
    out = np.empty((B, S, D), np.float32)
    for b in range(B):
        out[b] = res.results[2 * b]["out"] + res.results[2 * b + 1]["out"] + cvec
    return out


# revision 17
# speedup vs baseline: 1.2474x; 1.2474x over previous
"""MultiHeadAttention TRN2 Bass kernel.

Problem: B=4, S=2048, D=768, H=12 heads (DK=64).
Sharding: 8 cores = (batch b in 0..3) x (head-half in 0..1); each core
computes 6 heads of one batch element end-to-end (tensor-parallel over
heads within a batch). Host pre-transposes activations to [D, S] (and
casts to bf16 in the default fast path), slices projection weights per
head-half, and sums the two partial outputs per batch (+ bv@Wo + bo
correction, exact because softmax rows sum to 1).

On-core math:
  qh^T[384, S]: lhsT=Wq tile [Din,dout], rhs=q^T tile [Din,s] (+bq in drain)
  kh^T likewise; vh natural [S, 390] via lhsT=v^T tile, rhs=Wv:
    vh_aug[s, 65j..65j+64] = [m(s)*vh_head_j(s, :), m(s)]  (mask fold)
  S^T[k, q] = kh_head^T.T @ qh_head^T  (contraction d=64)
  P^T = exp(S^T * 0.125)               (ACT, fused scale, no max-sub)
  ctx_aug^T[0:65, q] += vh_aug_j[kc].T @ P^T[kc]  over k-chunks
    rows 0..63 = unnormalized ctx^T, row 64 = softmax denominator
  rs = approx-recip(denom); bcast via ones[1,64] K=1 f32r matmul;
  cn = ctx^T * rs   (drains deferred one (h,qb) iteration so the PE
                     pipeline never waits on the reciprocal chain)
  out[q, 768] = sum_dt cn[dt].T @ Wo tiles  (per 128-q chunk)
"""

import os
import sys
import types
from contextlib import ExitStack

import ml_dtypes
import numpy as np

import concourse.bacc as bacc
import concourse.bass as bass
import concourse.mybir as mybir
import concourse.tile as tile
from concourse import bass_utils
from concourse.bass import ts, ds

F32 = mybir.dt.float32
F32R = mybir.dt.float32r
BF16 = mybir.dt.bfloat16

D = 768        # model dim
DH = 384       # per-core head dim (6 heads x 64)
HPC = 6        # heads per core
VW = HPC * 65  # vh_aug free width (390)
QB = 1024      # q-block width in attention phase


def build_nc(S=2048, bf16=True):
    nc = bacc.Bacc("TRN2", target_bir_lowering=False, debug=False)

    MMD = BF16 if bf16 else F32R    # matmul operand dtype
    QBW = min(QB, S)                # attention q-block width
    NKT = S // 128                  # 128-wide s/k tiles
    NQB = S // QBW                  # q blocks
    NSC = S // 512                  # 512-wide s chunks

    qT = nc.dram_tensor("qT", [D, S], MMD, kind="ExternalInput").ap()
    kT = nc.dram_tensor("kT", [D, S], MMD, kind="ExternalInput").ap()
    vT = nc.dram_tensor("vT", [D, S], MMD, kind="ExternalInput").ap()
    wq = nc.dram_tensor("wq", [D, DH], MMD, kind="ExternalInput").ap()
    wk = nc.dram_tensor("wk", [D, DH], MMD, kind="ExternalInput").ap()
    wv = nc.dram_tensor("wv", [D, DH], MMD, kind="ExternalInput").ap()
    wo = nc.dram_tensor("wo", [DH, D], MMD, kind="ExternalInput").ap()
    bq = nc.dram_tensor("bq", [DH, 1], F32, kind="ExternalInput").ap()
    bk = nc.dram_tensor("bk", [DH, 1], F32, kind="ExternalInput").ap()
    mv = nc.dram_tensor("mv", [S, 1], F32, kind="ExternalInput").ap()
    out = nc.dram_tensor("out", [S, D], F32, kind="ExternalOutput").ap()

    with tile.TileContext(nc) as tc, ExitStack() as ctx:
        P = 128
        wpool = ctx.enter_context(tc.tile_pool(name="w", bufs=1))
        xin = ctx.enter_context(tc.tile_pool(name="xin", bufs=8))
        persist = ctx.enter_context(tc.tile_pool(name="persist", bufs=1))
        ppool = ctx.enter_context(tc.tile_pool(name="p", bufs=3))
        small = ctx.enter_context(tc.tile_pool(name="small", bufs=2))
        outp = ctx.enter_context(tc.tile_pool(name="outp", bufs=2))
        psA = ctx.enter_context(tc.tile_pool(name="psA", bufs=2, space="PSUM"))
        psB = ctx.enter_context(tc.tile_pool(name="psB", bufs=2, space="PSUM"))

        # ---- constants / small tensors ----
        wq_sb = [wpool.tile([P, DH], MMD, name=f"wq{c}", tag=f"wq{c}") for c in range(6)]
        wk_sb = [wpool.tile([P, DH], MMD, name=f"wk{c}", tag=f"wk{c}") for c in range(6)]
        wv_sb = [wpool.tile([P, DH], MMD, name=f"wv{c}", tag=f"wv{c}") for c in range(6)]
        wo_sb = [wpool.tile([P, D], MMD, name=f"wo{c}", tag=f"wo{c}") for c in range(3)]
        for c in range(6):
            (nc.sync if c % 2 == 0 else nc.gpsimd).dma_start(
                wk_sb[c][:], wk[ts(c, P), :]
            )
        bq_sb = [wpool.tile([P, 1], F32, name=f"bq{t}", tag=f"bq{t}") for t in range(3)]
        bk_sb = [wpool.tile([P, 1], F32, name=f"bk{t}", tag=f"bk{t}") for t in range(3)]
        for t in range(3):
            nc.sync.dma_start(bq_sb[t][:], bq[ts(t, P), :])
            nc.sync.dma_start(bk_sb[t][:], bk[ts(t, P), :])
        mv_sb = [wpool.tile([P, 1], F32, name=f"mv{st}", tag=f"mv{st}") for st in range(NKT)]
        for st in range(NKT):
            nc.sync.dma_start(mv_sb[st][:], mv[ts(st, P), :])
        ones6 = wpool.tile([P, HPC], F32, tag="ones6")
        nc.vector.memset(ones6[:], 1.0)
        ones64f = wpool.tile([1, 64], F32, tag="ones64f")
        nc.vector.memset(ones64f[:], 1.0)
        ones64 = wpool.tile([1, 64], F32R, tag="ones64")
        nc.vector.tensor_copy(ones64[:], ones64f[:])

        # ---- persistent activations ----
        khT = [persist.tile([P, S], MMD, name=f"khT{t}", tag=f"khT{t}") for t in range(3)]
        qhT = [persist.tile([P, S], MMD, name=f"qhT{t}", tag=f"qhT{t}") for t in range(3)]
        vh = [persist.tile([P, VW], MMD, name=f"vh{st}", tag=f"vh{st}") for st in range(NKT)]
        cn = [persist.tile([P, S], MMD, name=f"cn{t}", tag=f"cn{t}") for t in range(3)]

        # ---- phase 1a/1b: projections (q-proj of the second q-block is
        # deferred into the attention stream as PE filler work) ----
        def proj_chunk(xdram, wsb, bsb, dst, sc, pools=("psA", "psA", "psA")):
            xt = [xin.tile([P, QBW], MMD, name="xin", tag="xin") for c in range(6)]
            for c in range(6):
                (nc.sync if c % 2 == 0 else nc.gpsimd).dma_start(
                    xt[c][:], xdram[ts(c, P), ts(sc, QBW)]
                )
            for dt in range(3):
                pool = psA if pools[dt] == "psA" else psB
                ps = pool.tile([P, QBW], F32, name="prps", tag=pools[dt])
                for u in range(QBW // 512):
                    for c in range(6):
                        nc.tensor.matmul(
                            ps[:, ts(u, 512)],
                            lhsT=wsb[c][:, ts(dt, P)],
                            rhs=xt[c][:, ts(u, 512)],
                            start=(c == 0),
                            stop=(c == 5),
                        )
                nc.vector.tensor_scalar_add(
                    out=dst[dt][:, ts(sc, QBW)], in0=ps[:],
                    scalar1=bsb[dt][:],
                )

        # k-projection (alternating psum pools for deeper rotation)
        for sc in range(S // QBW):
            proj_chunk(kT, wk_sb, bk_sb, khT, sc,
                       pools=("psA", "psB", "psA") if sc % 2 == 0 else ("psB", "psA", "psB"))

        # v-projection
        for c in range(6):
            nc.sync.dma_start(wv_sb[c][:], wv[ts(c, P), :])
        for stb in range(0, NKT, QBW // 128):
            vt = [xin.tile([P, QBW], MMD, name="xin", tag="xin") for c in range(6)]
            for c in range(6):
                nc.sync.dma_start(vt[c][:], vT[ts(c, P), ds(stb * 128, QBW)])
            for sj in range(QBW // 128):
                st = stb + sj
                pool, ptag = (psA, "psA") if sj % 2 == 0 else (psB, "psB")
                ps = pool.tile([P, QBW], F32, name="vps", tag=ptag)
                for c in range(6):
                    nc.tensor.matmul(
                        ps[:, :DH],
                        lhsT=vt[c][:, ts(sj, P)],
                        rhs=wv_sb[c][:],
                        start=(c == 0),
                        stop=(c == 5),
                    )
                vh3 = vh[st].rearrange("p (h c) -> p h c", c=65)
                nc.vector.tensor_scalar_mul(
                    out=vh3[:, :, 0:64],
                    in0=ps[:, :DH].rearrange("p (h c) -> p h c", c=64),
                    scalar1=mv_sb[st][:],
                )
                nc.vector.tensor_scalar_mul(
                    out=vh3[:, :, 64:65],
                    in0=ones6[:].rearrange("p (h c) -> p h c", c=1),
                    scalar1=mv_sb[st][:],
                )

        # q-projection: first q-block now, rest deferred into phase 2
        for c in range(6):
            nc.sync.dma_start(wq_sb[c][:], wq[ts(c, P), :])
        proj_chunk(qT, wq_sb, bq_sb, qhT, 0,
                   pools=("psA", "psB", "psA"))
        for c in range(3):
            nc.sync.dma_start(wo_sb[c][:], wo[ts(c, P), :])
        pend_qproj = [(sc, dt) for sc in range(1, S // QBW) for dt in range(3)]
        qproj_xt = {}
        for sc in range(1, S // QBW):
            qproj_xt[sc] = [
                xin.tile([P, QBW], MMD, name="xin", tag="xin") for c in range(6)
            ]
            for c in range(6):
                (nc.sync if c % 2 == 0 else nc.gpsimd).dma_start(
                    qproj_xt[sc][c][:], qT[ts(c, P), ts(sc, QBW)]
                )

        # ---- phase 2: attention ----
        # Flat global pipeline over (h, qb, kc) steps: scores run 2 steps
        # ahead of attn@V; N=1024 single matmuls keep PE duty high so the
        # HAM clock stays unthrottled. Drains (recip + gpsimd broadcast)
        # are deferred ~5us so the PE never waits on them.
        hq = [(h, qb) for qb in range(NQB) for h in range(HPC)]
        steps = [(h, qb, kc) for (h, qb) in hq for kc in range(NKT)]

        ctx_ps = {}
        st_ps = {}

        def scores(h, qb, kc):
            dt, pb = h // 2, 64 * (h % 2)
            ps = psA.tile([P, QBW], F32, name="psA", tag="psA")
            for u in range(QBW // 512):
                nc.tensor.matmul(
                    ps[:, ts(u, 512)],
                    lhsT=khT[dt][pb : pb + 64, ts(kc, P)],
                    rhs=qhT[dt][pb : pb + 64, ds(qb * QBW + u * 512, 512)],
                    start=True,
                    stop=True,
                )
            st_ps[(h, qb, kc)] = ps

        def attnv(h, qb, kc, pt):
            for u in range(QBW // 512):
                nc.tensor.matmul(
                    ctx_ps[(h, qb)][0:65, ts(u, 512)],
                    lhsT=vh[kc][:, ds(65 * h, 65)],
                    rhs=pt[:, ts(u, 512)],
                    start=(kc == 0),
                    stop=(kc == NKT - 1),
                )

        def drain(h, qb):
            """Normalize + store ctx for a finished (h, qb)."""
            dt, pb = h // 2, 64 * (h % 2)
            cps = ctx_ps.pop((h, qb))
            rs = small.tile([1, QBW], F32, name="rs", tag="rs")
            with nc.allow_low_precision(reason="softmax denom recip"):
                nc.vector.reciprocal(rs[:], cps[64:65, :])
            bcs = small.tile([64, QBW], F32, name="bcs", tag="bcs")
            nc.gpsimd.partition_broadcast(bcs[:], rs[:])
            if pb == 0:
                nc.vector.tensor_tensor(
                    out=cn[dt][0:64, ts(qb, QBW)],
                    in0=cps[0:64, :],
                    in1=bcs[:],
                    op=mybir.AluOpType.mult,
                )
            else:
                tmp = small.tile([64, QBW], MMD, name="tmp", tag="tmp")
                nc.vector.tensor_tensor(
                    out=tmp[:], in0=cps[0:64, :], in1=bcs[:],
                    op=mybir.AluOpType.mult,
                )
                nc.sync.dma_start(cn[dt][64:128, ts(qb, QBW)], tmp[:])

        def oproj(qc):
            ups = psA.tile([P, 1024], F32, name="psA", tag="psA")
            for n0, nw in ((0, 512), (512, 256)):
                for dt in range(3):
                    nc.tensor.matmul(
                        ups[:, ds(n0, nw)],
                        lhsT=cn[dt][:, ts(qc, P)],
                        rhs=wo_sb[dt][:, ds(n0, nw)],
                        start=(dt == 0),
                        stop=(dt == 2),
                    )
            ot = outp.tile([P, D], F32, name="ot", tag="ot")
            nc.vector.tensor_copy(ot[:], ups[:, 0:D])
            nc.sync.dma_start(out[ts(qc, P), :], ot[:])

        DEPTH = 2
        pend_drain = []
        pend_oproj = []
        for n, (h, qb, kc) in enumerate(steps):
            if kc == 0:
                ctx_ps[(h, qb)] = psB.tile([P, QBW], F32, name="psB", tag="psB")[0:65, :]
            if n < DEPTH:
                scores(*steps[n])
            pt = ppool.tile([P, QBW], MMD, name="pt", tag="pt")
            nc.scalar.activation(
                pt[:], st_ps.pop((h, qb, kc))[:],
                mybir.ActivationFunctionType.Exp, scale=0.125,
            )
            if n + DEPTH < len(steps):
                scores(*steps[n + DEPTH])
            attnv(h, qb, kc, pt)
            if kc == 5 and pend_drain:
                hd, qd = pend_drain.pop(0)
                drain(hd, qd)
                if hd == HPC - 1:
                    pend_oproj.extend(range(qd * (QBW // P), (qd + 1) * (QBW // P)))
            elif pend_oproj and kc % 2 == 1 and kc != 5:
                oproj(pend_oproj.pop(0))
            elif pend_qproj and kc == 12:
                sc, dt = pend_qproj.pop(0)
                ps = psB.tile([P, QBW], F32, name="qprps", tag="psB")
                for u in range(QBW // 512):
                    for c in range(6):
                        nc.tensor.matmul(
                            ps[:, ts(u, 512)],
                            lhsT=wq_sb[c][:, ts(dt, P)],
                            rhs=qproj_xt[sc][c][:, ts(u, 512)],
                            start=(c == 0),
                            stop=(c == 5),
                        )
                nc.vector.tensor_scalar_add(
                    out=qhT[dt][:, ts(sc, QBW)], in0=ps[:],
                    scalar1=bq_sb[dt][:],
                )
                if dt == 2:
                    qproj_xt.pop(sc)
            if kc == NKT - 1:
                pend_drain.append((h, qb))
        for hd, qd in pend_drain:
            drain(hd, qd)
            if hd == HPC - 1:
                pend_oproj.extend(range(qd * (QBW // P), (qd + 1) * (QBW // P)))
        for qc in pend_oproj:
            oproj(qc)

    nc.compile()
    return nc


_NC_CACHE = {}


def _get_nc(S, bf16=True):
    key = (S, bf16)
    if key not in _NC_CACHE:
        _NC_CACHE[key] = build_nc(S, bf16)
    return _NC_CACHE[key]


def _install_ntff_hook():
    try:
        mod = types.ModuleType("antenv.axon_hooks")
        state = {"hook": None}
        mod.set_axon_ntff_profile_hook = lambda h: state.__setitem__("hook", h)
        mod.get_axon_ntff_profile_hook = lambda: state["hook"]
        sys.modules["antenv.axon_hooks"] = mod
        from trn_agent_boot.trn_boot import _ntff_profile_via_ctypes

        mod.set_axon_ntff_profile_hook(
            _ntff_profile_via_ctypes("/opt/axon/libaxon_pjrt.so")
        )
        bass_utils.upload_artifacts = lambda tmpdir: "local://" + tmpdir
        return state["hook"] is not None
    except Exception:
        return False


def run_cores(in_maps, S=2048, bf16=True, profile=False):
    nc = _get_nc(S, bf16)
    trace = bool(profile) and _install_ntff_hook()
    res = bass_utils.run_bass_kernel_spmd(
        nc, in_maps, core_ids=list(range(len(in_maps))), trace=trace
    )
    return res


def make_in_maps(q, k, v, mask, Wq, bq, Wk, bk, Wv, Wo, bf16=True):
    B = q.shape[0]
    mmd = ml_dtypes.bfloat16 if bf16 else np.float32
    qT = np.ascontiguousarray(
        np.asarray(q, np.float32).transpose(0, 2, 1)).astype(mmd)
    kT = np.ascontiguousarray(
        np.asarray(k, np.float32).transpose(0, 2, 1)).astype(mmd)
    vT = np.ascontiguousarray(
        np.asarray(v, np.float32).transpose(0, 2, 1)).astype(mmd)
    mvec = (~np.asarray(mask).reshape(B, -1)).astype(np.float32)
    Wq, Wk, Wv, Wo = (np.asarray(a, np.float32) for a in (Wq, Wk, Wv, Wo))
    bq, bk = np.asarray(bq, np.float32), np.asarray(bk, np.float32)
    in_maps = []
    for b in range(B):
        for half in range(2):
            hs = slice(DH * half, DH * (half + 1))
            in_maps.append(
                {
                    "qT": qT[b],
                    "kT": kT[b],
                    "vT": vT[b],
                    "wq": np.ascontiguousarray(Wq[:, hs]).astype(mmd),
                    "wk": np.ascontiguousarray(Wk[:, hs]).astype(mmd),
                    "wv": np.ascontiguousarray(Wv[:, hs]).astype(mmd),
                    "wo": np.ascontiguousarray(Wo[hs, :]).astype(mmd),
                    "bq": np.ascontiguousarray(bq[hs]).reshape(DH, 1),
                    "bk": np.ascontiguousarray(bk[hs]).reshape(DH, 1),
                    "mv": np.ascontiguousarray(mvec[b]).reshape(-1, 1),
                }
            )
    return in_maps


def kernel(q, k, v, mask, Wq, bq, Wk, bk, Wv, bv, Wo, bo):
    q = np.asarray(q, np.float32)
    B, S, _ = q.shape
    bf16 = os.environ.get("BASS_PRECISE") != "1"
    in_maps = make_in_maps(q, k, v, mask, Wq, bq, Wk, bk, Wv, Wo, bf16=bf16)
    res = run_cores(
        in_maps, S=S, bf16=bf16, profile=os.environ.get("BASS_PROFILE") == "1"
    )
    if os.environ.get("BASS_PROFILE") == "1" and res.exec_time_ns is not None:
        print(f"HW exec time: {res.exec_time_ns} ns")
    cvec = (
        np.asarray(bv, np.float32) @ np.asarray(Wo, np.float32)
        + np.asarray(bo, np.float32)
    )
    out = np.empty((B, S, D), np.float32)
    for b in range(B):
        out[b] = res.results[2 * b]["out"] + res.results[2 * b + 1]["out"] + cvec
    return out
